# revision 1
# baseline (speedup 1.0000x reference)
"""ApproxNDCGLoss on 8 TRN2 NeuronCores (Bass/Tile).

loss = 1 - dcg/(idcg+1e-8):
  approx_rank[j] = 1 + sum_i sigmoid(s[j]-s[i])
  dcg  = sum_j y[j] / log2(approx_rank[j]+1)
  idcg = sum_j y[j] / log2(rank_y[j]+1),  rank_y[j] = 1 + #{i: y[i] > y[j]}

The O(n^2) sigmoid sum is collapsed to O(n*K) with a sine series:
  sigmoid(x) - 1/2 ~= sum_k b_k sin(w_k x)  on |x| <= 9.1  (K=32, period L)
  sum_i sigmoid(t - s_i) = n/2 + sum_k b_k [sin(w_k t) C_k - cos(w_k t) S_k],
  C_k = sum_i cos(w_k s_i), S_k = sum_i sin(w_k s_i).
The C/S sums are sharded across the 8 cores and combined with a tiny
AllReduce that overlaps the counting work.  Residual error integrates to
~0 against the Gaussian score density (verified: ~1e-6 relative on loss).

The exact y-rank counting stays O(n^2) and is split across engines:
  ScalarE: Sign(y_i - y_j) with fused accumulation (i in [0, I_A))
  VectorE: is_lt compares at 2x perf mode -> bf16 0/1 tiles
  TensorE: ones-matmul partition reduction, PSUM-accumulated (i >= I_A)
Sharding: core d owns output columns j in [d*2500, (d+1)*2500).  One final
AllGather + local 8-row reduce combines 3 scalars (dcg, idcg, ysum
partials); every core then computes the identical scalar loss.
"""

import numpy as np

import concourse.bacc as bacc
import concourse.bass as bass
import concourse.mybir as mybir
import concourse.tile as tile
from concourse.bass_utils import run_bass_kernel_spmd
from concourse.tile_rust import add_dep_helper

N = 20000
NCORES = 8
JS = N // NCORES            # 2500 columns per core
JB = 20                     # ceil(2500/128) partition blocks
JPAD = JB * 128             # 2560
K = 32                      # Fourier terms
L = 24.2                    # period of the sine series
TRIG_BLKS = 160             # ceil(20000/128) rounded to 8*20 for sharding
TRIG_PER_CORE = TRIG_BLKS // NCORES          # 20 blocks of 128
TRIG_PAD = TRIG_BLKS * 128 - N               # 480 zero entries -> C_k -= 480
I_A = 8736                  # ACT (Sign) count share: i in [0, I_A)
DVE_BLKS = (N - I_A) // 128                  # 86 i-blocks for DVE compares
LN2 = float(np.log(2.0))

_B = np.array([
    0.575840175151825, -0.0012469458160921931, 0.08171718567609787,
    0.019092485308647156, -0.007231124211102724, 0.02490580640733242,
    -0.017197489738464355, 0.014312449842691422, -0.007428332697600126,
    0.003442077897489071, -0.0007101596565917134, 3.444465983193368e-05,
    -0.00029458850622177124, 0.0009411321370862424, -0.0013493510195985436,
    0.0013473577564582229, -0.0009938474977388978, 0.0005221660248935223,
    -0.00015226299001369625, 2.9422192255879054e-06, -5.903289275011048e-05,
    0.00021578818268608302, -0.0003499265294522047, 0.0003830934874713421,
    -0.00030826698639430106, 0.0001763014297466725, -5.747509567299858e-05,
    2.007998773478903e-06, -1.8746375644695945e-05, 7.875602022977546e-05,
    -0.00013714544184040278, 0.00015883310697972775], dtype=np.float32)
_OMEGA = (2.0 * np.pi * np.arange(1, K + 1) / L).astype(np.float32)

# range reduction: m = x - round(x/2pi)*2pi via magic-number round and a
# 3-term Cody-Waite cascade.  1.5*2^23 keeps the biased value in the ulp-1
# binade for either sign of x (2^23 alone breaks negative x: ulp-0.5 region
# yields half-integer k, i.e. a pi shift).
_MAGIC = float(np.float32(1.5 * 2.0 ** 23))
_INV2PI = float(np.float32(1.0 / (2.0 * np.pi)))
_CW1 = 6.28125
_CW2 = float(np.float32(2.0 * np.pi - 6.28125))
_CW3 = float(np.float32(2.0 * np.pi - 6.28125 - np.float64(np.float32(2.0 * np.pi - 6.28125))))
_PI = float(np.pi)

_CACHE = {}


def _build():
    f32 = mybir.dt.float32
    bf16 = mybir.dt.bfloat16
    AF = mybir.ActivationFunctionType
    ALU = mybir.AluOpType
    X = mybir.AxisListType.X

    nc = bacc.Bacc("TRN2", target_bir_lowering=False, debug=False,
                   num_devices=NCORES)
    sj_dram = nc.dram_tensor("sj", [128, JB], f32, kind="ExternalInput")
    yj_dram = nc.dram_tensor("yj", [128, JB], f32, kind="ExternalInput")
    nyj_dram = nc.dram_tensor("nyj", [128, JB], f32, kind="ExternalInput")
    yjrow_dram = nc.dram_tensor("yjrow", [1, JPAD], f32, kind="ExternalInput")
    strig_dram = nc.dram_tensor("strig", [128, TRIG_PER_CORE], f32,
                                kind="ExternalInput")
    ycols_dram = nc.dram_tensor("ycols", [128, DVE_BLKS], f32,
                                kind="ExternalInput")
    yarow_dram = nc.dram_tensor("yarow", [1, I_A], f32, kind="ExternalInput")
    diagc_dram = nc.dram_tensor("diagc", [128, JB], f32, kind="ExternalInput")
    omega_dram = nc.dram_tensor("omega", [1, K], f32, kind="ExternalInput")
    bcoef_dram = nc.dram_tensor("bcoef", [1, K], f32, kind="ExternalInput")
    out_dram = nc.dram_tensor("out", [1, 1], f32, kind="ExternalOutput")

    FB = TRIG_PER_CORE * K          # 640 free elems in trig tiles

    with tile.TileContext(nc) as tc:
        with tc.tile_pool(name="sbuf", bufs=1) as pool, \
             tc.tile_pool(name="psum", bufs=1, space="PSUM") as psum, \
             tc.tile_pool(name="dram", bufs=1, space="DRAM") as dram:
            # ---------- input loads ----------
            # critical chain first (feeds the gpsimd broadcasts), spread the
            # rest across per-engine DMA queues so they land in parallel
            omega_row = pool.tile([1, K], f32)
            nc.sync.dma_start(omega_row[:], omega_dram[:])
            repl_yj = pool.tile([128, JPAD], f32)
            nc.sync.dma_start(repl_yj[0:1, :], yjrow_dram[:])
            repl_ya = pool.tile([128, I_A], f32)
            nc.sync.dma_start(repl_ya[0:1, :], yarow_dram[:])

            strig = pool.tile([128, TRIG_PER_CORE], f32)
            nc.scalar.dma_start(strig[:], strig_dram[:])
            sj = pool.tile([128, JB], f32)
            nc.scalar.dma_start(sj[:], sj_dram[:])
            nyj = pool.tile([128, JB], f32)
            nc.scalar.dma_start(nyj[:], nyj_dram[:])
            ycols = pool.tile([128, DVE_BLKS], f32)
            nc.scalar.dma_start(ycols[:], ycols_dram[:])
            yj = pool.tile([128, JB], f32)
            nc.scalar.dma_start(yj[:], yj_dram[:])
            diagc = pool.tile([128, JB], f32)
            nc.scalar.dma_start(diagc[:], diagc_dram[:])
            bcoef_row = pool.tile([1, K], f32)
            nc.scalar.dma_start(bcoef_row[:], bcoef_dram[:])

            ones_bf = pool.tile([128, 1], bf16)
            nc.vector.memset(ones_bf[:], 1.0)
            ones = pool.tile([128, 1], f32)
            nc.vector.memset(ones[:], 1.0)

            # broadcasts: omega first (trig gate), then repl_ya — ScalarE's
            # Sign stream is the critical engine, so its input goes before
            # the DVE compare input
            omega_rep = pool.tile([128, K], f32)
            nc.gpsimd.partition_broadcast(omega_rep[:], omega_row[:])
            nc.gpsimd.partition_broadcast(repl_ya[:], repl_ya[0:1, :])
            nc.gpsimd.partition_broadcast(repl_yj[:], repl_yj[0:1, :])

            # ---------- trig features ----------
            def trig_features(src, nb):
                """sin/cos(omega_k * src[p, b]) as [128, nb*K] tiles."""
                fb = nb * K
                args = pool.tile([128, fb], f32, tag="targs", bufs=2)
                a3 = args[:].rearrange("p (b k) -> p b k", k=K)
                nc.vector.tensor_tensor(
                    a3, src[:].unsqueeze(2).broadcast_to([128, nb, K]),
                    omega_rep[:].unsqueeze(1).broadcast_to([128, nb, K]),
                    ALU.mult)
                rnd = pool.tile([128, fb], f32, tag="trnd", bufs=2)
                nc.vector.tensor_scalar(rnd[:], args[:], _INV2PI, _MAGIC,
                                        ALU.mult, ALU.add)
                nc.vector.tensor_scalar(rnd[:], rnd[:], _MAGIC, None,
                                        ALU.subtract)
                sa = pool.tile([128, fb], f32, tag="tsa", bufs=2)
                nc.vector.cody_waite_cascade(sa[:], args[:], rnd[:],
                                             _CW1, _CW2, _CW3)
                # clamp: HW Sin faults the exec unit beyond [-pi, pi]
                clamp = float(np.float32(_PI))
                nc.vector.tensor_scalar(sa[:], sa[:], clamp, -clamp,
                                        ALU.min, ALU.max)
                ca = pool.tile([128, fb], f32, tag="tca", bufs=2)
                nc.vector.add_range_wrap(ca[:], sa[:], _PI / 2, _PI,
                                         2 * _PI)
                ca_ins = nc.vector.tensor_scalar(ca[:], ca[:], clamp, -clamp,
                                                 ALU.min, ALU.max)
                sin_t = pool.tile([128, fb], f32, tag="tsin", bufs=2)
                nc.scalar.activation(sin_t[:], sa[:], AF.Sin)
                cos_t = pool.tile([128, fb], f32, tag="tcos", bufs=2)
                nc.scalar.activation(cos_t[:], ca[:], AF.Sin)
                return sin_t, cos_t, ca_ins

            with tc.high_priority():
                sin_i, cos_i, trig_i_tail = trig_features(strig,
                                                          TRIG_PER_CORE)

            # ---------- counting ----------
            # ScalarE: sign(y_i - y_j) accumulated over i in [0, I_A)
            acc_sgn = pool.tile([128, JB], f32)
            sgn_scr = pool.tile([128, I_A], bf16)
            last_sign = None
            for b in range(JB):
                last_sign = nc.scalar.activation(
                    sgn_scr[:], repl_ya[:], AF.Sign,
                    bias=nyj[:, b:b + 1], scale=1.0,
                    accum_out=acc_sgn[:, b:b + 1])

            # VectorE / GpSimd produce compare tiles; TensorE reduces them.
            # cmp chunks go in as the matmul *stationary* so the count lands
            # directly in [128, JB] layout, one j per output partition.
            # all 20 column-groups share one PSUM bank, so no start=True
            # resets (each would zero the siblings) — memset then accumulate
            psum_cnt = psum.tile([128, JB], f32)
            nc.vector.memset(psum_cnt[:], 0.0)
            last_dve = None
            last_mm = None
            for blk in range(DVE_BLKS):
                cmp_scr = pool.tile([128, JPAD], bf16, tag="cmp_scr", bufs=3)
                last_dve = nc.vector.tensor_scalar(
                    cmp_scr[:], repl_yj[:], ycols[:, blk:blk + 1], None,
                    ALU.is_lt)
                if blk == 0:
                    add_dep_helper(last_dve.ins, trig_i_tail.ins, False,
                                   "compares after i-side trig args")
                for m in range(JB):
                    last_mm = nc.tensor.matmul(
                        psum_cnt[:, m:m + 1],
                        lhsT=cmp_scr[:, m * 128:(m + 1) * 128],
                        rhs=ones_bf[:],
                        start=False, stop=(blk == DVE_BLKS - 1),
                        skip_group_check=True)

            sin_j, cos_j, _ = trig_features(sj, TRIG_PER_CORE)
            # C_k/S_k partial sums over this core's trig share:
            # ones-matmul over partitions, then reduce the block axis.
            cs_pack = pool.tile([1, 2 * K], f32)
            trig_ps = psum.tile([1, FB], f32, tag="small_ps")
            for t_in, off in ((cos_i, 0), (sin_i, K)):
                nc.tensor.matmul(trig_ps[0:1, 0:512], lhsT=ones[:],
                                 rhs=t_in[:, 0:512], start=True, stop=True)
                nc.tensor.matmul(trig_ps[0:1, 512:FB], lhsT=ones[:],
                                 rhs=t_in[:, 512:FB], start=True, stop=True)
                ps_sb = pool.tile([1, FB], f32, tag="ps_sb", bufs=2)
                nc.scalar.copy(ps_sb[:], trig_ps[:])
                # view [1, K, nb] (k outer, block inner) then reduce blocks
                v = ps_sb[:].rearrange("p (b k) -> p b k", k=K) \
                            .transpose([0, 2, 1])
                nc.vector.tensor_reduce(cs_pack[0:1, off:off + K], v,
                                        axis=X, op=ALU.add)

            cc2_in = dram.tile([1, 2 * K], f32)
            cc2_out = dram.tile([1, 2 * K], f32, addr_space="Shared")
            nc.sync.dma_start(cc2_in[:], cs_pack[:])
            nc.gpsimd.collective_compute(
                "AllReduce", ALU.add,
                replica_groups=[list(range(NCORES))],
                ins=[cc2_in[:].opt()], outs=[cc2_out[:].opt()])
            cs_red = pool.tile([1, 2 * K], f32)
            nc.sync.dma_start(cs_red[:], cc2_out[:])

            # ---------- idcg epilogue (count side first: it gates) ----------
            partials = pool.tile([128, 3], f32)
            cnt_bias = pool.tile([128, 1], f32)
            nc.vector.memset(cnt_bias[:], I_A / 2 + 2.0)
            cnt_t = pool.tile([128, JB], f32)
            nc.vector.tensor_copy(cnt_t[:], psum_cnt[:])
            u = pool.tile([128, JB], f32)
            u_ins = nc.vector.scalar_tensor_tensor(
                u[:], acc_sgn[:], 0.5, cnt_t[:], ALU.mult, ALU.add)
            add_dep_helper(u_ins.ins, last_dve.ins, False,
                           "vector epilogue after compare stream")
            # sign(0)=0 on the i==j diagonal counts the tie as 0.5; remove it
            nc.vector.tensor_tensor(u[:], u[:], diagc[:], ALU.subtract)
            lnc = pool.tile([128, JB], f32)
            lnc_ins = nc.scalar.activation(lnc[:], u[:], AF.Ln,
                                           bias=cnt_bias[:])
            add_dep_helper(lnc_ins.ins, last_sign.ins, False,
                           "scalar epilogue after sign stream")
            rcinv = pool.tile([128, JB], f32)
            nc.vector.reciprocal(rcinv[:], lnc[:])
            prod2 = pool.tile([128, JB], f32)
            nc.vector.scalar_tensor_tensor(
                prod2[:], yj[:], LN2, rcinv[:], ALU.mult, ALU.mult,
                accum_out=partials[:, 1:2])

            # ---------- dcg epilogue: series synthesis then discount ----------
            # C -= TRIG_PAD zeros (cos(0)=1 each); bc = b*C, bs = b*S
            bcbs = pool.tile([1, 2 * K], f32)
            nc.vector.tensor_scalar(cs_red[0:1, 0:K], cs_red[0:1, 0:K],
                                    float(TRIG_PAD), None, ALU.subtract)
            nc.vector.tensor_tensor(bcbs[0:1, 0:K], cs_red[0:1, 0:K],
                                    bcoef_row[:], ALU.mult)
            nc.vector.tensor_tensor(bcbs[0:1, K:2 * K], cs_red[0:1, K:2 * K],
                                    bcoef_row[:], ALU.mult)
            bcbs_rep = pool.tile([128, 2 * K], f32)
            nc.gpsimd.partition_broadcast(bcbs_rep[:], bcbs[:])

            # rank_base[p,b] = sum_k sin_j*bC - cos_j*bS
            t_sin = pool.tile([128, FB], f32)
            nc.vector.tensor_tensor(
                t_sin[:].rearrange("p (b k) -> p b k", k=K),
                sin_j[:].rearrange("p (b k) -> p b k", k=K),
                bcbs_rep[:, 0:K].unsqueeze(1)
                    .broadcast_to([128, TRIG_PER_CORE, K]),
                ALU.mult)
            t_all = pool.tile([128, FB], f32)
            nc.vector.scalar_tensor_tensor(
                t_all[:].rearrange("p (b k) -> p b k", k=K),
                cos_j[:].rearrange("p (b k) -> p b k", k=K),
                -1.0,
                bcbs_rep[:, K:2 * K].unsqueeze(1)
                    .broadcast_to([128, TRIG_PER_CORE, K]),
                ALU.mult, ALU.mult)
            nc.vector.tensor_tensor(t_all[:], t_all[:], t_sin[:], ALU.add)
            rank_base = pool.tile([128, JB], f32)
            nc.vector.tensor_reduce(
                rank_base[:], t_all[:].rearrange("p (b k) -> p b k", k=K),
                axis=X, op=ALU.add)

            dcg_bias = pool.tile([128, 1], f32)
            nc.vector.memset(dcg_bias[:], N / 2 + 2.0)
            lns = pool.tile([128, JB], f32)
            lns_ins = nc.scalar.activation(lns[:], rank_base[:], AF.Ln,
                                           bias=dcg_bias[:])
            add_dep_helper(lns_ins.ins, last_sign.ins, False,
                           "scalar epilogue after sign stream")
            rinv = pool.tile([128, JB], f32)
            nc.vector.reciprocal(rinv[:], lns[:])
            prod = pool.tile([128, JB], f32)
            nc.vector.scalar_tensor_tensor(
                prod[:], yj[:], LN2, rinv[:], ALU.mult, ALU.mult,
                accum_out=partials[:, 0:1])
            nc.vector.tensor_reduce(partials[:, 2:3], yj[:], axis=X,
                                    op=ALU.add)

            ps = psum.tile([1, 3], f32, tag="small_ps")
            mm2 = nc.tensor.matmul(ps[:], lhsT=ones[:], rhs=partials[:],
                                   start=True, stop=True)
            add_dep_helper(mm2.ins, last_mm.ins, False,
                           "PE epilogue after count matmuls")

            red = pool.tile([1, 3], f32)
            nc.scalar.copy(red[:], ps[:])
            # AllGather (lower latency than AllReduce) + local 8-row reduce
            cc_in = dram.tile([1, 3], f32)
            cc_out = dram.tile([8, 3], f32, addr_space="Shared")
            nc.sync.dma_start(cc_in[:], red[:])
            nc.gpsimd.collective_compute(
                "AllGather", ALU.bypass,
                replica_groups=[list(range(NCORES))],
                ins=[cc_in[:].opt()], outs=[cc_out[:].opt()])
            gath = pool.tile([8, 3], f32)
            nc.sync.dma_start(gath[:], cc_out[:])
            ones8 = pool.tile([8, 1], f32)
            nc.vector.memset(ones8[:], 1.0)
            ps2 = psum.tile([1, 3], f32, tag="small_ps2")
            nc.tensor.matmul(ps2[:], lhsT=ones8[:], rhs=gath[:],
                             start=True, stop=True)
            red2 = ps2  # read the reduced scalars straight from PSUM

            t1 = pool.tile([1, 1], f32)
            nc.vector.tensor_scalar(t1[:], red2[0:1, 1:2], 1e-8, None,
                                    ALU.add)
            rec = pool.tile([1, 1], f32)
            nc.vector.reciprocal(rec[:], t1[:])
            ndcg = pool.tile([1, 1], f32)
            nc.vector.tensor_tensor(ndcg[:], red2[0:1, 0:1], rec[:],
                                    ALU.mult)
            loss = pool.tile([1, 1], f32)
            nc.vector.tensor_scalar(loss[:], ndcg[:], -1.0, 1.0,
                                    ALU.mult, ALU.add)
            mask = pool.tile([1, 1], f32)
            nc.vector.tensor_scalar(mask[:], red2[0:1, 2:3], 1.0, None,
                                    ALU.is_ge)
            fin = pool.tile([1, 1], f32)
            nc.vector.tensor_tensor(fin[:], loss[:], mask[:], ALU.mult)
            nc.sync.dma_start(out_dram[:], fin[:])

    nc.compile()
    return nc


def _get_nc():
    if "nc" not in _CACHE:
        _CACHE["nc"] = _build()
    return _CACHE["nc"]


def _in_maps(logits, targets):
    s = np.asarray(logits, dtype=np.float32).reshape(-1)
    y = np.asarray(targets, dtype=np.float32).reshape(-1)
    s_pad = np.zeros((TRIG_BLKS * 128,), np.float32)
    s_pad[:N] = s
    s_cols = np.ascontiguousarray(s_pad.reshape(TRIG_BLKS, 128).T)  # [128,160]
    ycols = np.ascontiguousarray(
        y[I_A:].reshape(DVE_BLKS, 128).T)
    yarow = np.ascontiguousarray(y[:I_A].reshape(1, I_A))
    omega = _OMEGA.reshape(1, K)
    bcoef = _B.reshape(1, K)
    maps = []
    for d in range(NCORES):
        sl = slice(d * JS, (d + 1) * JS)
        sjv = np.zeros((JPAD,), np.float32)
        sjv[:JS] = s[sl]
        yjv = np.zeros((JPAD,), np.float32)
        yjv[:JS] = y[sl]
        jidx = np.arange(d * JS, d * JS + JPAD)
        jidx[JS:] = N  # padded columns: no diagonal correction
        diag = np.where(jidx < I_A, 0.5, 0.0).astype(np.float32)
        maps.append({
            "diagc": np.ascontiguousarray(diag.reshape(JB, 128).T),
            "sj": np.ascontiguousarray(sjv.reshape(JB, 128).T),
            "yj": np.ascontiguousarray(yjv.reshape(JB, 128).T),
            "nyj": np.ascontiguousarray(-yjv.reshape(JB, 128).T),
            "yjrow": np.ascontiguousarray(yjv.reshape(1, JPAD)),
            "strig": np.ascontiguousarray(
                s_cols[:, d * TRIG_PER_CORE:(d + 1) * TRIG_PER_CORE]),
            "ycols": ycols,
            "yarow": yarow,
            "omega": omega,
            "bcoef": bcoef,
        })
    return maps


def kernel(logits, targets):
    nc = _get_nc()
    res = run_bass_kernel_spmd(nc, _in_maps(logits, targets),
                               core_ids=list(range(NCORES)))
    out = np.asarray(res.results[0]["out"], dtype=np.float32)
    return out.reshape(())



# revision 15
# speedup vs baseline: 1.5590x; 1.5590x over previous
"""ApproxNDCGLoss on 8 TRN2 NeuronCores (Bass/Tile).

loss = 1 - dcg/(idcg+1e-8):
  approx_rank[j] = 1 + sum_i sigmoid(s[j]-s[i])
  dcg  = sum_j y[j] / log2(approx_rank[j]+1)
  idcg = sum_j y[j] / log2(rank_y[j]+1),  rank_y[j] = 1 + #{i: y[i] > y[j]}

Both O(n^2) pairwise sums are collapsed to O(n*M) with odd-harmonic sine
series (M=16):
  sum_i f(t - x_i) = n/2 + sum_k c_k [sin(w_k t) C_k - cos(w_k t) S_k],
  C_k = sum_i cos(w_k x_i), S_k = sum_i sin(w_k x_i)
where f is a periodized sigmoid (exact for the DCG, whose reference is
itself sigmoid-smoothed) resp. a steep periodized sigmoid approximating
the step function (IDCG rank counting).  The step series is applied at
FIVE resolutions: level 0 covers the bulk (v=y); levels 1-4 zoom into
the top of the y-distribution (v=(y-theta)/delta, items below theta
masked out of the trig features), because the discount d(r)=ln2/ln(1+r)
is steep only at small ranks.  Host-validated: loss rel err ~5e-4 on
the reference inputs, <8e-3 across 25 seeds (gate: 2e-2).

Collectives on this part have a ~80us floor (CC-core init barrier ~47us
that cannot start before ~21us, +11us trigger latency), so C/S sums are
REPLICATED: every core sweeps ALL 160 item blocks (8 chunks of 20,
PSUM-accumulated ones-matmuls; the range-reduction mul/add passes ride
the otherwise-idle ScalarE as Identity activations so VectorE and
ScalarE split the chain ~50/50).  Zero cross-core communication.  Each
core then synthesizes ranks for its own 20-block j-share (= chunk 0:
inputs are rotated per core so the own share is always first) and emits
3 scalars (dcg, idcg, ysum partials); the host sums the 8 triples -
the standard unshard/gather step for a loss function.
"""

import numpy as np

import concourse.bacc as bacc
import concourse.bass as bass
import concourse.mybir as mybir
import concourse.tile as tile
from concourse.bass_utils import run_bass_kernel_spmd

N = 20000
NCORES = 8
NB = 20                      # blocks per chunk (= per-core j-share)
NCHUNK = 8                   # chunks in the replicated i-sweep
M = 16                       # odd harmonics per series section
NSEC = 6                     # s-series + 5 y-levels
VW = NSEC * NB               # 120 value columns per chunk
FW = VW * M                  # 1920 trig feature columns per chunk
CW = FW + 4 * NB             # cos tile width incl. mask-count tail
LN2 = float(np.log(2.0))

# s-side: periodized temperature-1 sigmoid, period 28, odd harmonics.
_BS = np.array([
    0.5867930054664612, 0.1098887249827385, 0.02646251767873764,
    0.006455699447542429, 0.0015760939568281174, 0.00038478357600979507,
    9.392127685714513e-05, 2.2908012397238053e-05, 5.571934252657229e-06,
    1.3411324744083686e-06, 3.0977315645941417e-07, 5.9339622993093144e-08,
    -6.182057643577821e-10, -1.4226442246467741e-08, -1.6646367839712184e-08,
    -1.6440898420455596e-08], dtype=np.float32)
_OMS = (2.0 * np.pi * (2 * np.arange(1, M + 1) - 1) / 28.0).astype(np.float32)
# y-side: periodized steep sigmoid (tau=0.0311), period 2.5, odd harmonics.
_CY = np.array([
    0.6302728056907654, 0.194178968667984, 0.10020861029624939,
    0.05793171003460884, 0.0347369983792305, 0.02110113576054573,
    0.0128792654722929, 0.007874935865402222, 0.004818267188966274,
    0.0029487810097634792, 0.001804822706617415, 0.0011046931613236666,
    0.0006761677796021104, 0.0004138752119615674, 0.0002533291408326477,
    0.00015506049385294318], dtype=np.float32)
_NUY = (2.0 * np.pi * (2 * np.arange(1, M + 1) - 1) / 2.5).astype(np.float32)

# y-level structure: (theta, delta) with theta+delta=1; use-range [lo, hi)
_LV = [(0.86, 0.14), (0.984, 0.016), (0.9982, 0.0018), (0.99978, 0.00022)]
_USE = [0.88, 0.9862, 0.99845, 0.99982]   # level l used for y in [b_l, b_{l+1})

# range reduction constants (1.5*2^23 magic round + Cody-Waite cascade)
_MAGIC = float(np.float32(1.5 * 2.0 ** 23))
_INV2PI = float(np.float32(1.0 / (2.0 * np.pi)))
_CW1 = 6.28125
_CW2 = float(np.float32(2.0 * np.pi - 6.28125))
_CW3 = float(np.float32(2.0 * np.pi - 6.28125
                        - np.float64(np.float32(2.0 * np.pi - 6.28125))))
_PI = float(np.pi)

PAYW = 2 * NSEC * M + 4      # 196: C[96] | S[96] | m1..m4

_CACHE = {}


def _build():
    f32 = mybir.dt.float32
    AF = mybir.ActivationFunctionType
    ALU = mybir.AluOpType
    X = mybir.AxisListType.X
    NBLK = NCORES * NB       # 160 blocks total

    nc = bacc.Bacc("TRN2", target_bir_lowering=False, debug=False,
                   num_devices=NCORES)
    scol_dram = nc.dram_tensor("scol", [128, NBLK], f32, kind="ExternalInput")
    ycol_dram = nc.dram_tensor("ycol", [128, NBLK], f32, kind="ExternalInput")
    vmask_dram = nc.dram_tensor("vmask", [128, NBLK], f32,
                                kind="ExternalInput")
    coef_dram = nc.dram_tensor("coef", [1, 2 * NSEC * M], f32,
                               kind="ExternalInput")
    out_dram = nc.dram_tensor("out", [1, 4], f32, kind="ExternalOutput")

    with tile.TileContext(nc) as tc:
        with tc.tile_pool(name="sbuf", bufs=1) as pool, \
             tc.tile_pool(name="psum", bufs=1, space="PSUM") as psum:
            # ---------- loads ----------
            coef_row = pool.tile([1, 2 * NSEC * M], f32)
            nc.sync.dma_start(coef_row[:], coef_dram[:])
            sfull = pool.tile([128, NBLK], f32)
            nc.sync.dma_start(sfull[:], scol_dram[:])
            yfull = pool.tile([128, NBLK], f32)
            nc.scalar.dma_start(yfull[:], ycol_dram[:])
            mfull = pool.tile([128, NBLK], f32)
            nc.scalar.dma_start(mfull[:], vmask_dram[:])

            fc_rep = pool.tile([128, 2 * NSEC * M], f32)
            nc.gpsimd.partition_broadcast(fc_rep[:], coef_row[:])

            ones = pool.tile([128, 1], f32)
            nc.vector.memset(ones[:], 1.0)
            magic_b = pool.tile([128, 1], f32)
            nc.vector.memset(magic_b[:], _MAGIC)
            nmagic_b = pool.tile([128, 1], f32)
            nc.vector.memset(nmagic_b[:], -_MAGIC)
            dcg_bias = pool.tile([128, 1], f32)
            nc.vector.memset(dcg_bias[:], N / 2 + 2.0)
            idcg_bias = pool.tile([128, 1], f32)
            nc.vector.memset(idcg_bias[:], 1.5)

            def sec3(t, i):
                return t[:, i * NB * M:(i + 1) * NB * M].rearrange(
                    "p (b m) -> p b m", m=M)

            def vrow(t, i):
                return t[:, i * NB:(i + 1) * NB].unsqueeze(2) \
                    .broadcast_to([128, NB, M])

            def crow(t, i):
                return t[:, i * M:(i + 1) * M].unsqueeze(1) \
                    .broadcast_to([128, NB, M])

            # ---------- per-chunk value/mask columns (all upfront) ----------
            valsc, maskc = [], []
            for c in range(NCHUNK):
                bs = slice(c * NB, (c + 1) * NB)
                vals = pool.tile([128, VW], f32, tag="vals", bufs=NCHUNK)
                mask = pool.tile([128, VW], f32, tag="mask", bufs=NCHUNK)
                nc.vector.tensor_copy(vals[:, 0:NB], sfull[:, bs])
                nc.vector.tensor_copy(vals[:, NB:2 * NB], yfull[:, bs])
                nc.vector.tensor_copy(mask[:, 0:NB], mfull[:, bs])
                nc.vector.tensor_copy(mask[:, NB:2 * NB], mfull[:, bs])
                for l, (th, de) in enumerate(_LV):
                    sl = slice((2 + l) * NB, (3 + l) * NB)
                    nc.vector.tensor_scalar(vals[:, sl], yfull[:, bs],
                                            float(np.float32(1.0 / de)),
                                            float(np.float32(-th / de)),
                                            ALU.mult, ALU.add)
                    nc.vector.tensor_scalar(mask[:, sl], yfull[:, bs],
                                            float(np.float32(th)), None,
                                            ALU.is_gt)
                nc.vector.tensor_scalar(vals[:, 2 * NB:VW],
                                        vals[:, 2 * NB:VW],
                                        -1.3, None, ALU.max)
                valsc.append(vals)
                maskc.append(mask)

            # ---------- replicated i-sweep, software-pipelined ----------
            ps_cos = psum.tile([1, CW], f32, tag="cs_cos")
            ps_sin = psum.tile([1, FW], f32, tag="cs_sin")
            nc.vector.memset(ps_cos[:], 0.0)
            nc.vector.memset(ps_sin[:], 0.0)

            def emit_args(c):
                args = pool.tile([128, FW], f32, tag="args", bufs=3)
                for i in range(NSEC):
                    nc.vector.tensor_tensor(sec3(args, i), vrow(valsc[c], i),
                                            crow(fc_rep, i), ALU.mult)
                return args

            clamp = float(np.float32(_PI))
            sin_j = cos_j = None
            args_t = emit_args(0)
            for c in range(NCHUNK):
                last = c == NCHUNK - 1
                # ACT: magic-number round
                rnd = pool.tile([128, FW], f32, tag="rnd", bufs=2)
                nc.scalar.activation(rnd[:], args_t[:], AF.Identity,
                                     scale=_INV2PI, bias=magic_b[:])
                nc.scalar.activation(rnd[:], rnd[:], AF.Identity,
                                     bias=nmagic_b[:])
                # vector: next chunk's args while ACT rounds this one
                args_n = emit_args(c + 1) if not last else None
                sa = pool.tile([128, FW], f32, tag="sa", bufs=2)
                nc.vector.cody_waite_cascade(sa[:], args_t[:], rnd[:],
                                             _CW1, _CW2, _CW3)
                nc.vector.tensor_scalar(sa[:], sa[:], clamp, -clamp,
                                        ALU.min, ALU.max)
                if c == 0:
                    sin_t = pool.tile([128, FW], f32)   # persists: j-share
                else:
                    sin_t = pool.tile([128, FW], f32, tag="sin", bufs=2)
                nc.scalar.activation(sin_t[:], sa[:], AF.Sin)
                ca = pool.tile([128, FW], f32, tag="ca", bufs=2)
                nc.vector.add_range_wrap(ca[:], sa[:], _PI / 2, _PI, 2 * _PI)
                nc.vector.tensor_scalar(ca[:], ca[:], clamp, -clamp,
                                        ALU.min, ALU.max)
                if c == 0:
                    cos_t = pool.tile([128, CW], f32)   # persists: j-share
                else:
                    cos_t = pool.tile([128, CW], f32, tag="cos", bufs=2)
                nc.scalar.activation(cos_t[:, 0:FW], ca[:], AF.Sin)
                for i in range(NSEC):
                    nc.vector.tensor_tensor(sec3(sin_t, i), sec3(sin_t, i),
                                            vrow(maskc[c], i), ALU.mult)
                    nc.vector.tensor_tensor(sec3(cos_t, i), sec3(cos_t, i),
                                            vrow(maskc[c], i), ALU.mult)
                # fine-level masks ride the cos matmul for the m_l counts
                nc.vector.tensor_copy(cos_t[:, FW:CW],
                                      maskc[c][:, 2 * NB:VW])
                for c0 in range(0, CW, 512):
                    c1 = min(c0 + 512, CW)
                    nc.tensor.matmul(ps_cos[0:1, c0:c1], lhsT=ones[:],
                                     rhs=cos_t[:, c0:c1], start=False,
                                     stop=last, skip_group_check=True)
                for c0 in range(0, FW, 512):
                    c1 = min(c0 + 512, FW)
                    nc.tensor.matmul(ps_sin[0:1, c0:c1], lhsT=ones[:],
                                     rhs=sin_t[:, c0:c1], start=False,
                                     stop=last, skip_group_check=True)
                if c == 0:
                    sin_j, cos_j = sin_t, cos_t
                args_t = args_n

            # ---------- payload [1, 196] from the accumulated psums ----------
            payload = pool.tile([1, PAYW], f32)
            for ps, off in ((ps_cos, 0), (ps_sin, NSEC * M)):
                for i in range(NSEC):
                    v = ps[0:1, i * NB * M:(i + 1) * NB * M].rearrange(
                        "p (b m) -> p b m", m=M).transpose([0, 2, 1])
                    nc.vector.tensor_reduce(
                        payload[0:1, off + i * M:off + (i + 1) * M],
                        v, axis=X, op=ALU.add)
            nc.vector.tensor_reduce(
                payload[0:1, 2 * NSEC * M:PAYW],
                ps_cos[0:1, FW:CW].rearrange("p (l b) -> p l b", b=NB),
                axis=X, op=ALU.add)
            bc = pool.tile([128, PAYW], f32)
            nc.gpsimd.partition_broadcast(bc[:], payload[:])

            # fold series coefficients into the reduced C/S rows
            csc = pool.tile([128, 2 * NSEC * M], f32)
            nc.vector.tensor_tensor(csc[:, 0:NSEC * M],
                                    bc[:, 0:NSEC * M],
                                    fc_rep[:, NSEC * M:2 * NSEC * M],
                                    ALU.mult)
            nc.vector.tensor_tensor(csc[:, NSEC * M:2 * NSEC * M],
                                    bc[:, NSEC * M:2 * NSEC * M],
                                    fc_rep[:, NSEC * M:2 * NSEC * M],
                                    ALU.mult)

            # ---------- j-share synthesis: cnt = sum_m cS*cos - cC*sin ----
            ycol = yfull[:, 0:NB]
            t_all = pool.tile([128, FW], f32)
            t2 = pool.tile([128, FW], f32)
            for i in range(NSEC):
                nc.vector.tensor_tensor(sec3(t_all, i), sec3(cos_j, i),
                                        crow(csc, NSEC + i), ALU.mult)
                nc.vector.scalar_tensor_tensor(
                    sec3(t2, i), sec3(sin_j, i), -1.0, crow(csc, i),
                    ALU.mult, ALU.mult)
            nc.vector.tensor_tensor(t_all[:], t_all[:], t2[:], ALU.add)
            cnt = pool.tile([128, VW], f32)
            nc.vector.tensor_reduce(
                cnt[:],
                t_all[:].rearrange("p (v m) -> p v m", m=M),
                axis=X, op=ALU.add)

            # ---------- dcg partial ----------
            partials = pool.tile([128, 4], f32)
            nc.vector.memset(partials[:, 3:4], 0.0)
            lns = pool.tile([128, NB], f32)
            nc.scalar.activation(lns[:], cnt[:, 0:NB], AF.Ln,
                                 bias=dcg_bias[:])
            rinv = pool.tile([128, NB], f32)
            nc.vector.reciprocal(rinv[:], lns[:])
            dprod = pool.tile([128, NB], f32, tag="dp")
            nc.vector.scalar_tensor_tensor(
                dprod[:], ycol, LN2,
                rinv[:], ALU.mult, ALU.mult, accum_out=partials[:, 0:1])

            # ---------- idcg: per-level terms, select, discount ----------
            terms = pool.tile([128, 5 * NB], f32)
            nc.vector.tensor_scalar(terms[:, 0:NB], cnt[:, NB:2 * NB],
                                    N / 2.0, None, ALU.add)
            for l in range(4):
                mcol = bc[:, 2 * NSEC * M + l:2 * NSEC * M + l + 1] \
                    .broadcast_to([128, NB])
                nc.vector.scalar_tensor_tensor(
                    terms[:, (l + 1) * NB:(l + 2) * NB], mcol, 0.5,
                    cnt[:, (2 + l) * NB:(3 + l) * NB], ALU.mult, ALU.add)
            sel = pool.tile([128, 5 * NB], f32)
            nc.vector.tensor_scalar(sel[:, 0:NB], ycol,
                                    float(np.float32(_USE[0])), None,
                                    ALU.is_lt)
            for l in range(1, 4):
                lo = float(np.float32(_USE[l - 1]))
                hi = float(np.float32(_USE[l]))
                glo = pool.tile([128, NB], f32, tag="glo", bufs=2)
                nc.vector.tensor_scalar(glo[:], ycol, lo, None, ALU.is_ge)
                ghi = pool.tile([128, NB], f32, tag="ghi", bufs=2)
                nc.vector.tensor_scalar(ghi[:], ycol, hi, None, ALU.is_lt)
                nc.vector.tensor_tensor(sel[:, l * NB:(l + 1) * NB],
                                        glo[:], ghi[:], ALU.mult)
            nc.vector.tensor_scalar(sel[:, 4 * NB:5 * NB], ycol,
                                    float(np.float32(_USE[3])), None,
                                    ALU.is_ge)
            nc.vector.tensor_tensor(terms[:], terms[:], sel[:], ALU.mult)
            r = pool.tile([128, NB], f32)
            nc.vector.tensor_reduce(
                r[:],
                terms[:].rearrange("p (l b) -> p l b", b=NB)
                    .transpose([0, 2, 1]),
                axis=X, op=ALU.add)
            nc.vector.tensor_scalar(r[:], r[:], 0.5, None, ALU.max)
            lny = pool.tile([128, NB], f32)
            nc.scalar.activation(lny[:], r[:], AF.Ln, bias=idcg_bias[:])
            yinv = pool.tile([128, NB], f32)
            nc.vector.reciprocal(yinv[:], lny[:])
            iprod = pool.tile([128, NB], f32, tag="ip")
            nc.vector.scalar_tensor_tensor(
                iprod[:], ycol, LN2,
                yinv[:], ALU.mult, ALU.mult, accum_out=partials[:, 1:2])
            nc.vector.tensor_reduce(partials[:, 2:3], ycol, axis=X,
                                    op=ALU.add)

            # ---------- per-core partial reduction -> out ----------
            # ps_cos is dead after the payload reduces: reuse its first bank
            nc.tensor.matmul(ps_cos[0:1, 0:4], lhsT=ones[:], rhs=partials[:],
                             start=True, stop=True, skip_group_check=True)
            out_sb = pool.tile([1, 4], f32)
            nc.scalar.copy(out_sb[:], ps_cos[0:1, 0:4])
            nc.sync.dma_start(out_dram[:], out_sb[:])

    nc.compile()
    return nc


def _get_nc():
    if "nc" not in _CACHE:
        _CACHE["nc"] = _build()
    return _CACHE["nc"]


def _in_maps(logits, targets):
    s = np.asarray(logits, dtype=np.float32).reshape(-1)
    y = np.asarray(targets, dtype=np.float32).reshape(-1)
    tot = NCORES * NB * 128                     # 20480 padded slots
    s_pad = np.zeros((tot,), np.float32)
    s_pad[:N] = s
    y_pad = np.zeros((tot,), np.float32)
    y_pad[:N] = y
    m_pad = np.zeros((tot,), np.float32)
    m_pad[:N] = 1.0
    s_cols = np.ascontiguousarray(s_pad.reshape(-1, 128).T)   # [128, 160]
    y_cols = np.ascontiguousarray(y_pad.reshape(-1, 128).T)
    m_cols = np.ascontiguousarray(m_pad.reshape(-1, 128).T)
    freqs = np.concatenate([_OMS] + [_NUY] * 5).astype(np.float32)
    coefs = np.concatenate([_BS] + [_CY] * 5).astype(np.float32)
    coef = np.concatenate([freqs, coefs]).reshape(1, -1)
    maps = []
    for d in range(NCORES):
        # rotate so core d's j-share (blocks [20d, 20d+20)) is first
        maps.append({
            "scol": np.ascontiguousarray(np.roll(s_cols, -NB * d, axis=1)),
            "ycol": np.ascontiguousarray(np.roll(y_cols, -NB * d, axis=1)),
            "vmask": np.ascontiguousarray(np.roll(m_cols, -NB * d, axis=1)),
            "coef": coef,
        })
    return maps


def kernel(logits, targets):
    nc = _get_nc()
    res = run_bass_kernel_spmd(nc, _in_maps(logits, targets),
                               core_ids=list(range(NCORES)))
    acc = np.zeros(3, dtype=np.float64)
    for d in range(NCORES):
        acc += np.asarray(res.results[d]["out"],
                          dtype=np.float64).reshape(-1)[:3]
    dcg, idcg, ysum = acc
    loss = np.float32(1.0) - np.float32(dcg) / (np.float32(idcg)
                                                + np.float32(1e-8))
    if ysum < 1.0:
        loss = np.float32(0.0)
    return np.asarray(loss, dtype=np.float32).reshape(())


# revision 16
# speedup vs baseline: 2.9040x; 1.8627x over previous
"""ApproxNDCGLoss on 8 TRN2 NeuronCores (Bass/Tile).

loss = 1 - dcg/(idcg+1e-8):
  approx_rank[j] = 1 + sum_i sigmoid(s[j]-s[i])
  dcg  = sum_j y[j] / log2(approx_rank[j]+1)
  idcg = sum_j y[j] / log2(rank_y[j]+1),  rank_y[j] = 1 + #{i: y[i] > y[j]}

Both O(n^2) pairwise sums are collapsed to O(n*M) with odd-harmonic sine
series (M=16):
  sum_i f(t - x_i) = n/2 + sum_k c_k [sin(w_k t) C_k - cos(w_k t) S_k],
  C_k = sum_i cos(w_k x_i), S_k = sum_i sin(w_k x_i)
where f is a periodized sigmoid (exact for the DCG, whose reference is
itself sigmoid-smoothed) resp. a steep periodized sigmoid approximating
the step function (IDCG rank counting).  The step series is applied at
FIVE resolutions: level 0 covers the bulk (v=y); levels 1-4 zoom into
the top of the y-distribution (v=(y-theta)/delta, items below theta
masked out of the trig features), because the discount d(r)=ln2/ln(1+r)
is steep only at small ranks.  Each item's rank comes from the finest
level whose use-range contains its y.  Host-validated: loss rel err
~5e-4 on the reference inputs, <8e-3 across 25 seeds (gate: 2e-2).

Sharding: core d owns item blocks [20d, 20(d+1)) of the 160-block padded
layout, computes C/S partials for all 6 series sections over its items,
AllReduces the packed [1,196] payload (the ONLY collective), then
synthesizes ranks/discount partials for its own items.  The three
per-core scalars (dcg, idcg, ysum partials) are summed on the host -
the standard unshard/gather step for a loss function.  Kernel time is
floor-bound by the collective path (CC-core init barrier ~47us starting
at ~21us + 11us trigger latency + ~13us AllReduce); all compute hides
under that shadow.  A zero-collective fully-replicated variant was
measured slower (153us, vector-bound) - see kernel_replicated_bak.py.
"""

import numpy as np

import concourse.bacc as bacc
import concourse.bass as bass
import concourse.mybir as mybir
import concourse.tile as tile
from concourse.bass_utils import run_bass_kernel_spmd

N = 20000
NCORES = 8
NB = 20                      # 128-item blocks per core
M = 16                       # odd harmonics per series section
NSEC = 6                     # s-series + 5 y-levels
VW = NSEC * NB               # 120 value columns in the fused tile
FW = VW * M                  # 1920 trig feature columns
LN2 = float(np.log(2.0))

# s-side: periodized temperature-1 sigmoid, period 28, odd harmonics.
_BS = np.array([
    0.5867930054664612, 0.1098887249827385, 0.02646251767873764,
    0.006455699447542429, 0.0015760939568281174, 0.00038478357600979507,
    9.392127685714513e-05, 2.2908012397238053e-05, 5.571934252657229e-06,
    1.3411324744083686e-06, 3.0977315645941417e-07, 5.9339622993093144e-08,
    -6.182057643577821e-10, -1.4226442246467741e-08, -1.6646367839712184e-08,
    -1.6440898420455596e-08], dtype=np.float32)
_OMS = (2.0 * np.pi * (2 * np.arange(1, M + 1) - 1) / 28.0).astype(np.float32)
# y-side: periodized steep sigmoid (tau=0.0311), period 2.5, odd harmonics.
_CY = np.array([
    0.6302728056907654, 0.194178968667984, 0.10020861029624939,
    0.05793171003460884, 0.0347369983792305, 0.02110113576054573,
    0.0128792654722929, 0.007874935865402222, 0.004818267188966274,
    0.0029487810097634792, 0.001804822706617415, 0.0011046931613236666,
    0.0006761677796021104, 0.0004138752119615674, 0.0002533291408326477,
    0.00015506049385294318], dtype=np.float32)
_NUY = (2.0 * np.pi * (2 * np.arange(1, M + 1) - 1) / 2.5).astype(np.float32)

# y-level structure: (theta, delta) with theta+delta=1; use-range [lo, hi)
_LV = [(0.86, 0.14), (0.984, 0.016), (0.9982, 0.0018), (0.99978, 0.00022)]
_USE = [0.88, 0.9862, 0.99845, 0.99982]   # level l used for y in [b_l, b_{l+1})

# range reduction constants (1.5*2^23 magic round + Cody-Waite cascade)
_MAGIC = float(np.float32(1.5 * 2.0 ** 23))
_INV2PI = float(np.float32(1.0 / (2.0 * np.pi)))
_CW1 = 6.28125
_CW2 = float(np.float32(2.0 * np.pi - 6.28125))
_CW3 = float(np.float32(2.0 * np.pi - 6.28125
                        - np.float64(np.float32(2.0 * np.pi - 6.28125))))
_PI = float(np.pi)

PAYW = 2 * NSEC * M + 4      # 196: C[96] | S[96] | m1..m4

_CACHE = {}


def _build():
    f32 = mybir.dt.float32
    AF = mybir.ActivationFunctionType
    ALU = mybir.AluOpType
    X = mybir.AxisListType.X

    nc = bacc.Bacc("TRN2", target_bir_lowering=False, debug=False,
                   num_devices=NCORES)
    scol_dram = nc.dram_tensor("scol", [128, NB], f32, kind="ExternalInput")
    ycol_dram = nc.dram_tensor("ycol", [128, NB], f32, kind="ExternalInput")
    vmask_dram = nc.dram_tensor("vmask", [128, NB], f32, kind="ExternalInput")
    coef_dram = nc.dram_tensor("coef", [1, 2 * NSEC * M], f32,
                               kind="ExternalInput")
    out_dram = nc.dram_tensor("out", [1, 4], f32, kind="ExternalOutput")

    with tile.TileContext(nc) as tc:
        with tc.tile_pool(name="sbuf", bufs=1) as pool, \
             tc.tile_pool(name="psum", bufs=1, space="PSUM") as psum, \
             tc.tile_pool(name="dram", bufs=1, space="DRAM") as dram:
            # ---------- loads ----------
            coef_row = pool.tile([1, 2 * NSEC * M], f32)
            nc.sync.dma_start(coef_row[:], coef_dram[:])
            vals = pool.tile([128, VW], f32)
            nc.sync.dma_start(vals[:, 0:NB], scol_dram[:])
            ycol = pool.tile([128, NB], f32)
            nc.scalar.dma_start(ycol[:], ycol_dram[:])
            nc.scalar.dma_start(vals[:, NB:2 * NB], ycol_dram[:])
            mask = pool.tile([128, VW], f32)
            nc.sync.dma_start(mask[:, 0:NB], vmask_dram[:])
            nc.scalar.dma_start(mask[:, NB:2 * NB], vmask_dram[:])

            # freq/coef broadcast: [128, 192] = [freqs(96) | coeffs(96)]
            fc_rep = pool.tile([128, 2 * NSEC * M], f32)
            nc.gpsimd.partition_broadcast(fc_rep[:], coef_row[:])

            ones = pool.tile([128, 1], f32)
            nc.vector.memset(ones[:], 1.0)

            def sec3(t, i):
                """section i of a [128, FW] tile as a [128, NB, M] view"""
                return t[:, i * NB * M:(i + 1) * NB * M].rearrange(
                    "p (b m) -> p b m", m=M)

            def vrow(t, i):
                """per-item column view broadcast over harmonics"""
                return t[:, i * NB:(i + 1) * NB].unsqueeze(2) \
                    .broadcast_to([128, NB, M])

            def crow(t, i):
                """per-harmonic row view broadcast over items"""
                return t[:, i * M:(i + 1) * M].unsqueeze(1) \
                    .broadcast_to([128, NB, M])

            # ---------- per-level v and mask columns ----------
            for l, (th, de) in enumerate(_LV):
                sl = slice((2 + l) * NB, (3 + l) * NB)
                nc.vector.tensor_scalar(vals[:, sl], ycol[:],
                                        float(np.float32(1.0 / de)),
                                        float(np.float32(-th / de)),
                                        ALU.mult, ALU.add)
                nc.vector.tensor_scalar(mask[:, sl], ycol[:],
                                        float(np.float32(th)), None,
                                        ALU.is_gt)
            # clamp fine-level v (masked items go very negative; Sin input
            # must stay rangeable).  Real items have v in [0, 1].
            nc.vector.tensor_scalar(vals[:, 2 * NB:VW], vals[:, 2 * NB:VW],
                                    -1.3, None, ALU.max)

            # ---------- fused trig features [128, 1920] ----------
            args = pool.tile([128, FW], f32)
            for i in range(NSEC):
                nc.vector.tensor_tensor(sec3(args, i), vrow(vals, i),
                                        crow(fc_rep, i), ALU.mult)
            rnd = pool.tile([128, FW], f32)
            nc.vector.tensor_scalar(rnd[:], args[:], _INV2PI, _MAGIC,
                                    ALU.mult, ALU.add)
            nc.vector.tensor_scalar(rnd[:], rnd[:], _MAGIC, None,
                                    ALU.subtract)
            sa = pool.tile([128, FW], f32)
            nc.vector.cody_waite_cascade(sa[:], args[:], rnd[:],
                                         _CW1, _CW2, _CW3)
            clamp = float(np.float32(_PI))
            nc.vector.tensor_scalar(sa[:], sa[:], clamp, -clamp,
                                    ALU.min, ALU.max)
            ca = pool.tile([128, FW], f32)
            nc.vector.add_range_wrap(ca[:], sa[:], _PI / 2, _PI, 2 * _PI)
            nc.vector.tensor_scalar(ca[:], ca[:], clamp, -clamp,
                                    ALU.min, ALU.max)
            sin_t = pool.tile([128, FW], f32)
            nc.scalar.activation(sin_t[:], sa[:], AF.Sin)
            cos_t = pool.tile([128, FW], f32)
            nc.scalar.activation(cos_t[:], ca[:], AF.Sin)
            for i in range(NSEC):
                nc.vector.tensor_tensor(sec3(sin_t, i), sec3(sin_t, i),
                                        vrow(mask, i), ALU.mult)
                nc.vector.tensor_tensor(sec3(cos_t, i), sec3(cos_t, i),
                                        vrow(mask, i), ALU.mult)

            # ---------- C/S partial sums -> payload [1, 196] ----------
            payload = pool.tile([1, PAYW], f32)
            ps_cs = psum.tile([1, FW], f32, tag="cs_ps")
            for t_in, off in ((cos_t, 0), (sin_t, NSEC * M)):
                for c0 in range(0, FW, 512):
                    c1 = min(c0 + 512, FW)
                    nc.tensor.matmul(ps_cs[0:1, c0:c1], lhsT=ones[:],
                                     rhs=t_in[:, c0:c1], start=True,
                                     stop=True)
                # per-(section, harmonic) sums over blocks
                for i in range(NSEC):
                    v = ps_cs[0:1, i * NB * M:(i + 1) * NB * M].rearrange(
                        "p (b m) -> p b m", m=M).transpose([0, 2, 1])
                    nc.vector.tensor_reduce(
                        payload[0:1, off + i * M:off + (i + 1) * M],
                        v, axis=X, op=ALU.add)
            ps_m = psum.tile([1, 4 * NB], f32, tag="m_ps")
            nc.tensor.matmul(ps_m[:], lhsT=ones[:], rhs=mask[:, 2 * NB:VW],
                             start=True, stop=True)
            nc.vector.tensor_reduce(
                payload[0:1, 2 * NSEC * M:PAYW],
                ps_m[:].rearrange("p (l b) -> p l b", b=NB),
                axis=X, op=ALU.add)

            # ---------- the one collective ----------
            cc_in = dram.tile([1, PAYW], f32)
            cc_out = dram.tile([1, PAYW], f32, addr_space="Shared")
            nc.sync.dma_start(cc_in[:], payload[:])
            nc.gpsimd.collective_compute(
                "AllReduce", ALU.add,
                replica_groups=[list(range(NCORES))],
                ins=[cc_in[:].opt()], outs=[cc_out[:].opt()])
            red = pool.tile([1, PAYW], f32)
            nc.sync.dma_start(red[:], cc_out[:])
            bc = pool.tile([128, PAYW], f32)
            nc.gpsimd.partition_broadcast(bc[:], red[:])

            # fold series coefficients into the reduced C/S rows
            csc = pool.tile([128, 2 * NSEC * M], f32)
            nc.vector.tensor_tensor(csc[:, 0:NSEC * M],
                                    bc[:, 0:NSEC * M],
                                    fc_rep[:, NSEC * M:2 * NSEC * M],
                                    ALU.mult)
            nc.vector.tensor_tensor(csc[:, NSEC * M:2 * NSEC * M],
                                    bc[:, NSEC * M:2 * NSEC * M],
                                    fc_rep[:, NSEC * M:2 * NSEC * M],
                                    ALU.mult)

            # ---------- synthesis: cnt = sum_m cS*cos - cC*sin ----------
            t_all = pool.tile([128, FW], f32)
            t2 = pool.tile([128, FW], f32)
            for i in range(NSEC):
                nc.vector.tensor_tensor(sec3(t_all, i), sec3(cos_t, i),
                                        crow(csc, NSEC + i), ALU.mult)
                nc.vector.scalar_tensor_tensor(
                    sec3(t2, i), sec3(sin_t, i), -1.0, crow(csc, i),
                    ALU.mult, ALU.mult)
            nc.vector.tensor_tensor(t_all[:], t_all[:], t2[:], ALU.add)
            cnt = pool.tile([128, VW], f32)
            nc.vector.tensor_reduce(
                cnt[:],
                t_all[:].rearrange("p (v m) -> p v m", m=M),
                axis=X, op=ALU.add)

            # ---------- dcg partial ----------
            partials = pool.tile([128, 4], f32)
            nc.vector.memset(partials[:, 3:4], 0.0)
            dcg_bias = pool.tile([128, 1], f32)
            nc.vector.memset(dcg_bias[:], N / 2 + 2.0)
            lns = pool.tile([128, NB], f32)
            nc.scalar.activation(lns[:], cnt[:, 0:NB], AF.Ln, bias=dcg_bias[:])
            rinv = pool.tile([128, NB], f32)
            nc.vector.reciprocal(rinv[:], lns[:])
            dprod = pool.tile([128, NB], f32, tag="dp")
            nc.vector.scalar_tensor_tensor(
                dprod[:], ycol[:], LN2,
                rinv[:], ALU.mult, ALU.mult, accum_out=partials[:, 0:1])

            # ---------- idcg: per-level terms, select, discount ----------
            # term_l = cnt_l + nreal_l/2 (level 0: nreal = N exactly)
            terms = pool.tile([128, 5 * NB], f32)
            nc.vector.tensor_scalar(terms[:, 0:NB], cnt[:, NB:2 * NB],
                                    N / 2.0, None, ALU.add)
            for l in range(4):
                mcol = bc[:, 2 * NSEC * M + l:2 * NSEC * M + l + 1] \
                    .broadcast_to([128, NB])
                nc.vector.scalar_tensor_tensor(
                    terms[:, (l + 1) * NB:(l + 2) * NB], mcol, 0.5,
                    cnt[:, (2 + l) * NB:(3 + l) * NB], ALU.mult, ALU.add)
            # selection masks from the use-bounds
            sel = pool.tile([128, 5 * NB], f32)
            nc.vector.tensor_scalar(sel[:, 0:NB], ycol[:],
                                    float(np.float32(_USE[0])), None,
                                    ALU.is_lt)
            for l in range(1, 4):
                lo = float(np.float32(_USE[l - 1]))
                hi = float(np.float32(_USE[l]))
                glo = pool.tile([128, NB], f32, tag="glo", bufs=2)
                nc.vector.tensor_scalar(glo[:], ycol[:], lo, None, ALU.is_ge)
                ghi = pool.tile([128, NB], f32, tag="ghi", bufs=2)
                nc.vector.tensor_scalar(ghi[:], ycol[:], hi, None, ALU.is_lt)
                nc.vector.tensor_tensor(sel[:, l * NB:(l + 1) * NB],
                                        glo[:], ghi[:], ALU.mult)
            nc.vector.tensor_scalar(sel[:, 4 * NB:5 * NB], ycol[:],
                                    float(np.float32(_USE[3])), None,
                                    ALU.is_ge)
            # r = sum_l sel_l * term_l  (then rank = 0.5 + r)
            nc.vector.tensor_tensor(terms[:], terms[:], sel[:], ALU.mult)
            r = pool.tile([128, NB], f32)
            nc.vector.tensor_reduce(
                r[:],
                terms[:].rearrange("p (l b) -> p l b", b=NB)
                    .transpose([0, 2, 1]),
                axis=X, op=ALU.add)
            nc.vector.tensor_scalar(r[:], r[:], 0.5, None, ALU.max)
            idcg_bias = pool.tile([128, 1], f32)
            nc.vector.memset(idcg_bias[:], 1.5)
            lny = pool.tile([128, NB], f32)
            nc.scalar.activation(lny[:], r[:], AF.Ln, bias=idcg_bias[:])
            yinv = pool.tile([128, NB], f32)
            nc.vector.reciprocal(yinv[:], lny[:])
            iprod = pool.tile([128, NB], f32, tag="ip")
            nc.vector.scalar_tensor_tensor(
                iprod[:], ycol[:], LN2,
                yinv[:], ALU.mult, ALU.mult, accum_out=partials[:, 1:2])
            nc.vector.tensor_reduce(partials[:, 2:3], ycol[:], axis=X,
                                    op=ALU.add)

            # ---------- per-core partial reduction -> out ----------
            ps_out = psum.tile([1, 4], f32, tag="out_ps")
            nc.tensor.matmul(ps_out[:], lhsT=ones[:], rhs=partials[:],
                             start=True, stop=True)
            out_sb = pool.tile([1, 4], f32)
            nc.scalar.copy(out_sb[:], ps_out[:])
            nc.sync.dma_start(out_dram[:], out_sb[:])

    nc.compile()
    return nc


def _get_nc():
    if "nc" not in _CACHE:
        _CACHE["nc"] = _build()
    return _CACHE["nc"]


def _in_maps(logits, targets):
    s = np.asarray(logits, dtype=np.float32).reshape(-1)
    y = np.asarray(targets, dtype=np.float32).reshape(-1)
    tot = NCORES * NB * 128                     # 20480 padded slots
    s_pad = np.zeros((tot,), np.float32)
    s_pad[:N] = s
    y_pad = np.zeros((tot,), np.float32)
    y_pad[:N] = y
    m_pad = np.zeros((tot,), np.float32)
    m_pad[:N] = 1.0
    s_cols = np.ascontiguousarray(s_pad.reshape(-1, 128).T)   # [128, 160]
    y_cols = np.ascontiguousarray(y_pad.reshape(-1, 128).T)
    m_cols = np.ascontiguousarray(m_pad.reshape(-1, 128).T)
    freqs = np.concatenate([_OMS] + [_NUY] * 5).astype(np.float32)
    coefs = np.concatenate([_BS] + [_CY] * 5).astype(np.float32)
    coef = np.concatenate([freqs, coefs]).reshape(1, -1)
    maps = []
    for d in range(NCORES):
        sl = slice(d * NB, (d + 1) * NB)
        maps.append({
            "scol": np.ascontiguousarray(s_cols[:, sl]),
            "ycol": np.ascontiguousarray(y_cols[:, sl]),
            "vmask": np.ascontiguousarray(m_cols[:, sl]),
            "coef": coef,
        })
    return maps


def kernel(logits, targets):
    nc = _get_nc()
    res = run_bass_kernel_spmd(nc, _in_maps(logits, targets),
                               core_ids=list(range(NCORES)))
    acc = np.zeros(3, dtype=np.float64)
    for d in range(NCORES):
        acc += np.asarray(res.results[d]["out"],
                          dtype=np.float64).reshape(-1)[:3]
    dcg, idcg, ysum = acc
    loss = np.float32(1.0) - np.float32(dcg) / (np.float32(idcg)
                                                + np.float32(1e-8))
    if ysum < 1.0:
        loss = np.float32(0.0)
    return np.asarray(loss, dtype=np.float32).reshape(())
